# revision 32
# baseline (speedup 1.0000x reference)
"""Bass/Trainium2 kernel for nn_Attention_27874337751826.

GQA attention block (16 Q heads, 4 KV heads, head_dim 128, hidden 2048,
B=2, S=2048) with per-head RMSNorm on q/k, RoPE, tanh soft-cap 50, causal
softmax, and output projection.

Sharding: 8 cores = 2 batches x 4 KV groups. Each core handles one batch
element and one KV group (4 q heads + 1 kv head), computing a partial
output (its heads' slice of Wo rows); the host sums the 4 partials per
batch.

Per-core layout strategy (all d-major / "transposed"):
  - hsT [hidden, S] fed from host; projections produce qT/kT/vT [128, S]
    in PSUM via fp32r matmuls (1 cycle/row).
  - RMSNorm: sum(q^2) over head dim via ones-vector matmul; rstd =
    sqrt(reciprocal(mean+eps)); broadcast across partitions via an
    outer-product matmul (ones_row x rstd).
  - RoPE: rotate_half as a 128x128 rotation-matrix matmul (norm weights
    folded into the rotation matrix and cos tables host-side), then two
    DVE multiplies + add.
  - Scores computed transposed: scoresT[sk, sq] = kT_tile.T @ qT block,
    tanh softcap on ACT (scale folded), causal mask via gpsimd
    affine_select (-1e9 pre-exp), exp on ACT.
  - attnT[d, sq] += v_tile.T(probsT) via matmul with v (s-major, from PE
    transpose) as stationary; softmax denominators via ones-vector
    matmul accumulated alongside; normalize with reciprocal + broadcast.
  - Output projection: attnT slices as stationary, Wo rows streaming;
    partial out [S, hidden] written back.
"""

import ml_dtypes
import numpy as np

import concourse.mybir as mybir
import concourse.tile as tile
from concourse import bacc
from concourse.bass_utils import run_bass_kernel_spmd

NUM_HEADS = 16
NUM_KV_HEADS = 4
NUM_KV_GROUPS = 4
D = 128
HID = 2048
SOFT_CAP = 50.0
NORM_EPS = 1e-6
ROPE_BASE = 1000000.0

F32 = mybir.dt.float32
F32R = mybir.dt.float32r
BF16 = mybir.dt.bfloat16

_BUILD_CACHE = {}


def _build(S):
    """Build + compile the per-core kernel for sequence length S."""
    nT = HID // 128          # hidden contraction tiles
    nS = S // 128            # seq tiles of 128
    nQ = S // 512            # seq blocks of 512
    HQ = NUM_HEADS // NUM_KV_GROUPS  # q heads per core (4)
    scale_tanh = (D ** -0.5) / SOFT_CAP

    nc = bacc.Bacc("TRN2", target_bir_lowering=False, debug=False, num_devices=8)

    hsT_d = nc.dram_tensor("hsT", [HID, S], BF16, kind="ExternalInput")
    wq_d = nc.dram_tensor("wq", [HID, HQ * D], BF16, kind="ExternalInput")
    wk_d = nc.dram_tensor("wk", [HID, D], BF16, kind="ExternalInput")
    wv_d = nc.dram_tensor("wv", [HID, D], BF16, kind="ExternalInput")
    wo_d = nc.dram_tensor("wo", [HQ * D, HID], BF16, kind="ExternalInput")
    cosq_d = nc.dram_tensor("cosq", [D, S], F32, kind="ExternalInput")
    cosk_d = nc.dram_tensor("cosk", [D, S], F32, kind="ExternalInput")
    sin_d = nc.dram_tensor("sin", [D, S], F32, kind="ExternalInput")
    rwq_d = nc.dram_tensor("rwq", [D, D], F32R, kind="ExternalInput")
    rwk_d = nc.dram_tensor("rwk", [D, D], F32R, kind="ExternalInput")
    idn_d = nc.dram_tensor("idn", [D, D], BF16, kind="ExternalInput")
    onesc_d = nc.dram_tensor("onesc", [128, 1], BF16, kind="ExternalInput")
    out_d = nc.dram_tensor("out", [S, HID], F32, kind="ExternalOutput")

    with tile.TileContext(nc) as tc:
        with (
            tc.tile_pool(name="wpool", bufs=1) as wp,
            tc.tile_pool(name="big", bufs=1) as bg,
            tc.tile_pool(name="qnp", bufs=2) as qnp,
            tc.tile_pool(name="atp", bufs=2) as atp,
            tc.tile_pool(name="work", bufs=3) as wk_pool,
            tc.tile_pool(name="qcp", bufs=6) as qcp,
            tc.tile_pool(name="stat", bufs=1) as st_pool,
            tc.tile_pool(name="stat2", bufs=2) as st2_pool,
            tc.tile_pool(name="wide", bufs=2) as wd_pool,
            tc.tile_pool(name="psum", bufs=1, space="PSUM") as pp,
        ):
            # ---- resident weights / tables ----
            wq_sb = wp.tile([128, nT, HQ * D], BF16)
            nc.sync.dma_start(wq_sb[:], wq_d.rearrange("(t p) m -> p t m", p=128))
            wk_sb = wp.tile([128, nT, D], BF16)
            nc.sync.dma_start(wk_sb[:], wk_d.rearrange("(t p) m -> p t m", p=128))
            wv_sb = wp.tile([128, nT, D], BF16)
            nc.sync.dma_start(wv_sb[:], wv_d.rearrange("(t p) m -> p t m", p=128))
            wo_sb = wp.tile([128, HQ, HID], BF16)
            nc.sync.dma_start(wo_sb[:], wo_d.rearrange("(h p) m -> p h m", p=128))
            cosq_sb = wp.tile([D, S], F32)
            nc.sync.dma_start(cosq_sb[:], cosq_d[:])
            cosk_sb = wp.tile([D, S], F32)
            nc.sync.dma_start(cosk_sb[:], cosk_d[:])
            sin_sb = wp.tile([D, S], F32)
            nc.sync.dma_start(sin_sb[:], sin_d[:])
            rwq_sb = wp.tile([D, D], F32R)
            nc.sync.dma_start(rwq_sb[:], rwq_d[:])
            rwk_sb = wp.tile([D, D], F32R)
            nc.sync.dma_start(rwk_sb[:], rwk_d[:])
            idn_sb = wp.tile([D, D], BF16)
            nc.sync.dma_start(idn_sb[:], idn_d[:])
            ones_col = wp.tile([128, 1], BF16)
            nc.sync.dma_start(ones_col[:], onesc_d[:])

            # persistent activations
            kn = [bg.tile([D, 512], BF16, name=f"kn{q}") for q in range(nQ)]
            vv = [bg.tile([128, D], BF16, name=f"v{s}") for s in range(nS)]

            qn = {}
            at = {}

            passes = [
                [("q", 0), ("q", 1)],
                [("q", 2), ("q", 3)],
                [("k", 0), ("v", 0)],
            ]

            def w_slice(kind, idx, t):
                if kind == "q":
                    return wq_sb[:, t, idx * D:(idx + 1) * D]
                if kind == "k":
                    return wk_sb[:, t, :]
                return wv_sb[:, t, :]

            def rope_stats(Q, qsl, o, ps, qc_tiles, im, islice):
                """square/stats into im[islice]; rope ps -> qc_tiles[o]."""
                is_k = o == 4
                cos_sb = cosk_sb if is_k else cosq_sb
                rw_sb = rwk_sb if is_k else rwq_sb
                sq = wk_pool.tile([128, 512], BF16, tag="tmpa")
                nc.scalar.square(sq[:], ps[:], )
                ss = pp.tile([1, 512], F32, name="pm")
                nc.tensor.matmul(ss[:], ones_col[:], sq[:], start=True, stop=True)
                m = st2_pool.tile([1, 512], F32, tag="m")
                nc.vector.tensor_scalar(
                    m[:], ss[:], 1.0 / D, NORM_EPS,
                    mybir.AluOpType.mult, mybir.AluOpType.add,
                )
                scr = st2_pool.tile([1, 512], F32, tag="scr")
                nc.vector.reciprocal_approx_accurate(im[:, islice], m[:], scr[:])

                qtsb = wk_pool.tile([128, 512], F32R, tag="qtsb")
                nc.scalar.copy(qtsb[:], ps[:])
                qc = qcp.tile([128, 512], F32, tag="qc")
                qc_tiles[o] = qc
                nc.vector.tensor_tensor(
                    qc[:], ps[:], cos_sb[:, qsl], mybir.AluOpType.mult
                )
                rot = pp.tile([128, 512], F32, name="sa0")
                nc.tensor.matmul(rot[:], rw_sb[:], qtsb[:], start=True, stop=True)
                qs = wk_pool.tile([128, 512], F32, tag="tmpa")
                nc.vector.tensor_tensor(
                    qs[:], rot[:], sin_sb[:, qsl], mybir.AluOpType.mult
                )
                nc.vector.tensor_tensor(qc[:], qc[:], qs[:], mybir.AluOpType.add)

            def apply_norm(Q, o, rstd, islice, qc_tiles):
                bc = wk_pool.tile([128, 512], F32, tag="bcb")
                nc.gpsimd.partition_broadcast(bc[:], rstd[:, islice].bitcast(F32))
                if o == 4:
                    dst = kn[Q]
                else:
                    dst = qnp.tile([D, 512], BF16, tag=f"qn{o}")
                    qn[(o, Q)] = dst
                nc.vector.tensor_tensor(
                    dst[:], qc_tiles[o][:], bc[:], mybir.AluOpType.mult
                )

            def phase_P(Q):
                qsl = slice(Q * 512, (Q + 1) * 512)
                qc_tiles = {}
                im_q = st_pool.tile([1, 4 * 512], F32, tag="imq")
                im_k = st_pool.tile([1, 512], F32, tag="imk")
                for pidx, pdef in enumerate(passes):
                    proj = [pp.tile([128, 512], F32, name=f"pp{i}") for i in range(2)]
                    for t in range(nT):
                        hst = wk_pool.tile([128, 512], BF16, tag="hst")
                        nc.sync.dma_start(
                            hst[:], hsT_d[t * 128:(t + 1) * 128, qsl]
                        )
                        for i, (kind, idx) in enumerate(pdef):
                            nc.tensor.matmul(
                                proj[i][:], w_slice(kind, idx, t), hst[:],
                                start=(t == 0), stop=(t == nT - 1),
                            )
                    for i, (kind, idx) in enumerate(pdef):
                        ps = proj[i]
                        if kind == "v":
                            vtsb = wk_pool.tile([128, 512], BF16, tag="vtsb")
                            nc.vector.tensor_copy(vtsb[:], ps[:])
                            for st in range(4):
                                vt_ps = pp.tile([128, 128], BF16, name="pm")
                                nc.tensor.transpose(
                                    vt_ps[:], vtsb[:, st * 128:(st + 1) * 128],
                                    idn_sb[:],
                                )
                                nc.vector.tensor_copy(vv[Q * 4 + st][:], vt_ps[:])
                        elif kind == "k":
                            rope_stats(Q, qsl, 4, ps, qc_tiles, im_k, slice(0, 512))
                        else:
                            o = idx
                            rope_stats(
                                Q, qsl, o, ps, qc_tiles, im_q,
                                slice(o * 512, (o + 1) * 512),
                            )
                    if pidx == 1:
                        rstd_q = st_pool.tile([1, 4 * 512], F32R, tag="rstdq")
                        nc.scalar.sqrt(rstd_q[:], im_q[:])
                        for o in range(4):
                            apply_norm(Q, o, rstd_q, slice(o * 512, (o + 1) * 512), qc_tiles)
                    elif pidx == 2:
                        rstd_k = st_pool.tile([1, 512], F32R, tag="rstdk")
                        nc.scalar.sqrt(rstd_k[:], im_k[:])
                        apply_norm(Q, 4, rstd_k, slice(0, 512), qc_tiles)

            def phase_A_h(Q, h):
                last_sj = Q * 4 + 3
                att = pp.tile([128, 512], F32, name="sat")
                ssum = pp.tile([1, 512], F32, name="ssm")
                for sj in range(last_sj + 1):
                    sc = pp.tile(
                        [128, 512], F32,
                        name=("sa0" if sj % 2 == 0 else "sa1"),
                    )
                    nc.tensor.matmul(
                        sc[:],
                        kn[sj // 4][:, (sj % 4) * 128:(sj % 4 + 1) * 128],
                        qn[(h, Q)][:],
                        start=True, stop=True,
                    )
                    t_sb = wk_pool.tile([128, 512], BF16, tag="tt")
                    nc.scalar.activation(
                        t_sb[:], sc[:], mybir.ActivationFunctionType.Tanh,
                        scale=scale_tanh,
                    )
                    if sj >= Q * 4:
                        nc.gpsimd.affine_select(
                            out=t_sb[:], in_=t_sb[:],
                            compare_op=mybir.AluOpType.is_ge,
                            fill=-1e9,
                            base=Q * 512 - sj * 128,
                            pattern=[[1, 512]],
                            channel_multiplier=-1,
                        )
                    e_sb = wk_pool.tile([128, 512], BF16, tag="ee")
                    nc.scalar.activation(
                        e_sb[:], t_sb[:], mybir.ActivationFunctionType.Exp,
                        scale=SOFT_CAP,
                    )
                    nc.tensor.matmul(
                        att[:], vv[sj][:], e_sb[:],
                        start=(sj == 0), stop=(sj == last_sj),
                    )
                    nc.tensor.matmul(
                        ssum[:], ones_col[:], e_sb[:],
                        start=(sj == 0), stop=(sj == last_sj),
                    )
                sf = st2_pool.tile([1, 512], F32, tag="sf")
                nc.vector.tensor_copy(sf[:], ssum[:])
                scr2 = st2_pool.tile([1, 512], F32, tag="scr")
                rf = st2_pool.tile([1, 512], F32, tag="rf")
                nc.vector.reciprocal_approx_accurate(rf[:], sf[:], scr2[:])
                bcr_sb = wk_pool.tile([128, 512], F32, tag="evac")
                nc.gpsimd.partition_broadcast(bcr_sb[:], rf[:])
                at_t = atp.tile([D, 512], BF16, tag=f"at{h}")
                at[(h, Q)] = at_t
                nc.vector.tensor_tensor(
                    at_t[:], att[:], bcr_sb[:], mybir.AluOpType.mult
                )

            def phase_O(Q):
                for st in range(4):
                    row0 = Q * 512 + st * 128
                    for hb in range(4):
                        po = pp.tile([128, 512], F32, name="po")
                        for h in range(HQ):
                            nc.tensor.matmul(
                                po[:], at[(h, Q)][:, st * 128:(st + 1) * 128],
                                wo_sb[:, h, hb * 512:(hb + 1) * 512],
                                start=(h == 0), stop=(h == HQ - 1),
                            )
                        ob = wk_pool.tile([128, 512], F32, tag="evac")
                        nc.vector.tensor_copy(ob[:], po[:])
                        nc.sync.dma_start(
                            out_d[row0:row0 + 128, hb * 512:(hb + 1) * 512], ob[:]
                        )

            for Q in range(nQ):
                phase_P(Q)
                if Q > 0:
                    phase_O(Q - 1)
                for h in range(HQ):
                    phase_A_h(Q, h)
            phase_O(nQ - 1)

    nc.compile()
    return nc


def _get_nc(S):
    if S not in _BUILD_CACHE:
        _BUILD_CACHE[S] = _build(S)
    return _BUILD_CACHE[S]


def _rope_tables(S):
    inv_freq = 1.0 / (ROPE_BASE ** (np.arange(0, D, 2, dtype=np.float64) / D))
    pos = np.arange(S, dtype=np.float64)
    freqs = np.outer(pos, inv_freq)                  # [S, D/2]
    emb = np.concatenate([freqs, freqs], axis=-1)    # [S, D]
    return (
        np.cos(emb).T.astype(np.float32).copy(),     # [D, S]
        np.sin(emb).T.astype(np.float32).copy(),
    )


def _rot_matrix():
    R = np.zeros((D, D), dtype=np.float32)
    half = D // 2
    for i in range(half):
        R[i, i + half] = -1.0
        R[i + half, i] = 1.0
    return R


def run_sharded(hidden_states, Wq, Wk, Wv, Wo, q_norm_w, k_norm_w, trace=False):
    hidden_states = np.asarray(hidden_states, dtype=np.float32)
    Wq = np.asarray(Wq, dtype=np.float32)
    Wk = np.asarray(Wk, dtype=np.float32)
    Wv = np.asarray(Wv, dtype=np.float32)
    Wo = np.asarray(Wo, dtype=np.float32)
    q_norm_w = np.asarray(q_norm_w, dtype=np.float32)
    k_norm_w = np.asarray(k_norm_w, dtype=np.float32)

    B, S, _ = hidden_states.shape
    nc = _get_nc(S)

    cosT, sinT = _rope_tables(S)
    cosq = np.ascontiguousarray(cosT * q_norm_w[:, None])
    cosk = np.ascontiguousarray(cosT * k_norm_w[:, None])
    R = _rot_matrix()
    rwq = np.ascontiguousarray(R.T * q_norm_w[:, None])  # lhsT for rot-matmul
    rwk = np.ascontiguousarray(R.T * k_norm_w[:, None])
    idn = np.eye(D, dtype=np.float32)

    bf16 = ml_dtypes.bfloat16
    hsT = [np.ascontiguousarray(hidden_states[b].T).astype(bf16) for b in range(B)]

    in_maps = []
    for b in range(B):
        for g in range(NUM_KV_GROUPS):
            c0 = g * (NUM_HEADS // NUM_KV_GROUPS) * D
            c1 = (g + 1) * (NUM_HEADS // NUM_KV_GROUPS) * D
            in_maps.append({
                "hsT": hsT[b],
                "wq": np.ascontiguousarray(Wq[:, c0:c1]).astype(bf16),
                "wk": np.ascontiguousarray(Wk[:, g * D:(g + 1) * D]).astype(bf16),
                "wv": np.ascontiguousarray(Wv[:, g * D:(g + 1) * D]).astype(bf16),
                "wo": np.ascontiguousarray(Wo[c0:c1, :]).astype(bf16),
                "cosq": cosq,
                "cosk": cosk,
                "sin": sinT,
                "rwq": rwq,
                "rwk": rwk,
                "idn": idn.astype(bf16),
                "onesc": np.ones((128, 1), dtype=bf16),
            })

    res = run_bass_kernel_spmd(
        nc, in_maps, core_ids=list(range(len(in_maps))), trace=trace
    )

    out = np.zeros((B, S, HID), dtype=np.float64)
    for b in range(B):
        for g in range(NUM_KV_GROUPS):
            out[b] += res.results[b * NUM_KV_GROUPS + g]["out"].astype(np.float64)
    return out.astype(np.float32), res


def kernel(hidden_states, Wq, Wk, Wv, Wo, q_norm_w, k_norm_w):
    out, _ = run_sharded(hidden_states, Wq, Wk, Wv, Wo, q_norm_w, k_norm_w)
    return out


# revision 33
# speedup vs baseline: 1.0034x; 1.0034x over previous
"""Bass/Trainium2 kernel for nn_Attention_27874337751826.

GQA attention block (16 Q heads, 4 KV heads, head_dim 128, hidden 2048,
B=2, S=2048) with per-head RMSNorm on q/k, RoPE, tanh soft-cap 50, causal
softmax, and output projection.

Sharding: 8 cores = 2 batches x 4 KV groups. Each core handles one batch
element and one KV group (4 q heads + 1 kv head), computing a partial
output (its heads' slice of Wo rows); the host sums the 4 partials per
batch.

Per-core layout strategy (all d-major / "transposed"):
  - hsT [hidden, S] fed from host; projections produce qT/kT/vT [128, S]
    in PSUM via fp32r matmuls (1 cycle/row).
  - RMSNorm: sum(q^2) over head dim via ones-vector matmul; rstd =
    sqrt(reciprocal(mean+eps)); broadcast across partitions via an
    outer-product matmul (ones_row x rstd).
  - RoPE: rotate_half as a 128x128 rotation-matrix matmul (norm weights
    folded into the rotation matrix and cos tables host-side), then two
    DVE multiplies + add.
  - Scores computed transposed: scoresT[sk, sq] = kT_tile.T @ qT block,
    tanh softcap on ACT (scale folded), causal mask via gpsimd
    affine_select (-1e9 pre-exp), exp on ACT.
  - attnT[d, sq] += v_tile.T(probsT) via matmul with v (s-major, from PE
    transpose) as stationary; softmax denominators via ones-vector
    matmul accumulated alongside; normalize with reciprocal + broadcast.
  - Output projection: attnT slices as stationary, Wo rows streaming;
    partial out [S, hidden] written back.
"""

import ml_dtypes
import numpy as np

import concourse.mybir as mybir
import concourse.tile as tile
from concourse import bacc
from concourse.bass_utils import run_bass_kernel_spmd

NUM_HEADS = 16
NUM_KV_HEADS = 4
NUM_KV_GROUPS = 4
D = 128
HID = 2048
SOFT_CAP = 50.0
NORM_EPS = 1e-6
ROPE_BASE = 1000000.0

F32 = mybir.dt.float32
F32R = mybir.dt.float32r
BF16 = mybir.dt.bfloat16

_BUILD_CACHE = {}


def _build(S):
    """Build + compile the per-core kernel for sequence length S."""
    nT = HID // 128          # hidden contraction tiles
    nS = S // 128            # seq tiles of 128
    nQ = S // 512            # seq blocks of 512
    HQ = NUM_HEADS // NUM_KV_GROUPS  # q heads per core (4)
    scale_tanh = (D ** -0.5) / SOFT_CAP

    nc = bacc.Bacc("TRN2", target_bir_lowering=False, debug=False, num_devices=8)

    hsT_d = nc.dram_tensor("hsT", [HID, S], BF16, kind="ExternalInput")
    wq_d = nc.dram_tensor("wq", [HID, HQ * D], BF16, kind="ExternalInput")
    wk_d = nc.dram_tensor("wk", [HID, D], BF16, kind="ExternalInput")
    wv_d = nc.dram_tensor("wv", [HID, D], BF16, kind="ExternalInput")
    wo_d = nc.dram_tensor("wo", [HQ * D, HID], BF16, kind="ExternalInput")
    cosq_d = nc.dram_tensor("cosq", [D, S], F32, kind="ExternalInput")
    cosk_d = nc.dram_tensor("cosk", [D, S], F32, kind="ExternalInput")
    sin_d = nc.dram_tensor("sin", [D, S], F32, kind="ExternalInput")
    rwq_d = nc.dram_tensor("rwq", [D, D], F32R, kind="ExternalInput")
    rwk_d = nc.dram_tensor("rwk", [D, D], F32R, kind="ExternalInput")
    idn_d = nc.dram_tensor("idn", [D, D], BF16, kind="ExternalInput")
    onesc_d = nc.dram_tensor("onesc", [128, 1], BF16, kind="ExternalInput")
    out_d = nc.dram_tensor("out", [S, HID], F32, kind="ExternalOutput")

    with tile.TileContext(nc) as tc:
        with (
            tc.tile_pool(name="wpool", bufs=1) as wp,
            tc.tile_pool(name="big", bufs=1) as bg,
            tc.tile_pool(name="qnp", bufs=2) as qnp,
            tc.tile_pool(name="atp", bufs=2) as atp,
            tc.tile_pool(name="work", bufs=3) as wk_pool,
            tc.tile_pool(name="qcp", bufs=6) as qcp,
            tc.tile_pool(name="stat", bufs=1) as st_pool,
            tc.tile_pool(name="stat2", bufs=2) as st2_pool,
            tc.tile_pool(name="wide", bufs=2) as wd_pool,
            tc.tile_pool(name="psum", bufs=1, space="PSUM") as pp,
        ):
            # ---- resident weights / tables ----
            wq_sb = wp.tile([128, nT, HQ * D], BF16)
            nc.sync.dma_start(wq_sb[:], wq_d.rearrange("(t p) m -> p t m", p=128))
            wk_sb = wp.tile([128, nT, D], BF16)
            nc.sync.dma_start(wk_sb[:], wk_d.rearrange("(t p) m -> p t m", p=128))
            wv_sb = wp.tile([128, nT, D], BF16)
            nc.sync.dma_start(wv_sb[:], wv_d.rearrange("(t p) m -> p t m", p=128))
            wo_sb = wp.tile([128, HQ, HID], BF16)
            nc.sync.dma_start(wo_sb[:], wo_d.rearrange("(h p) m -> p h m", p=128))
            cosq_sb = wp.tile([D, S], F32)
            nc.sync.dma_start(cosq_sb[:], cosq_d[:])
            cosk_sb = wp.tile([D, S], F32)
            nc.sync.dma_start(cosk_sb[:], cosk_d[:])
            sin_sb = wp.tile([D, S], F32)
            nc.sync.dma_start(sin_sb[:], sin_d[:])
            rwq_sb = wp.tile([D, D], F32R)
            nc.sync.dma_start(rwq_sb[:], rwq_d[:])
            rwk_sb = wp.tile([D, D], F32R)
            nc.sync.dma_start(rwk_sb[:], rwk_d[:])
            idn_sb = wp.tile([D, D], BF16)
            nc.sync.dma_start(idn_sb[:], idn_d[:])
            ones_col = wp.tile([128, 1], BF16)
            nc.sync.dma_start(ones_col[:], onesc_d[:])

            # persistent activations
            kn = [bg.tile([D, 512], BF16, name=f"kn{q}") for q in range(nQ)]
            vv = [bg.tile([128, D], BF16, name=f"v{s}") for s in range(nS)]

            qn = {}
            at = {}

            passes = [
                [("q", 0), ("q", 1), ("q", 2)],
                [("q", 3), ("k", 0), ("v", 0)],
            ]

            def w_slice(kind, idx, t):
                if kind == "q":
                    return wq_sb[:, t, idx * D:(idx + 1) * D]
                if kind == "k":
                    return wk_sb[:, t, :]
                return wv_sb[:, t, :]

            def rope_stats(Q, qsl, o, ps, qc_tiles, im, islice):
                """square/stats into im[islice]; rope ps -> qc_tiles[o]."""
                is_k = o == 4
                cos_sb = cosk_sb if is_k else cosq_sb
                rw_sb = rwk_sb if is_k else rwq_sb
                sq = wk_pool.tile([128, 512], BF16, tag="tmpa")
                nc.scalar.square(sq[:], ps[:], )
                ss = pp.tile([1, 512], F32, name="pm")
                nc.tensor.matmul(ss[:], ones_col[:], sq[:], start=True, stop=True)
                m = st2_pool.tile([1, 512], F32, tag="m")
                nc.vector.tensor_scalar(
                    m[:], ss[:], 1.0 / D, NORM_EPS,
                    mybir.AluOpType.mult, mybir.AluOpType.add,
                )
                scr = st2_pool.tile([1, 512], F32, tag="scr")
                nc.vector.reciprocal_approx_accurate(im[:, islice], m[:], scr[:])

                qtsb = wk_pool.tile([128, 512], F32R, tag="qtsb")
                nc.scalar.copy(qtsb[:], ps[:])
                qc = qcp.tile([128, 512], F32, tag="qc")
                qc_tiles[o] = qc
                nc.vector.tensor_tensor(
                    qc[:], ps[:], cos_sb[:, qsl], mybir.AluOpType.mult
                )
                rot = pp.tile([128, 512], F32, name="sa0")
                nc.tensor.matmul(rot[:], rw_sb[:], qtsb[:], start=True, stop=True)
                qs = wk_pool.tile([128, 512], F32, tag="tmpa")
                nc.vector.tensor_tensor(
                    qs[:], rot[:], sin_sb[:, qsl], mybir.AluOpType.mult
                )
                nc.vector.tensor_tensor(qc[:], qc[:], qs[:], mybir.AluOpType.add)

            def apply_norm(Q, o, rstd, islice, qc_tiles):
                bc = wk_pool.tile([128, 512], F32, tag="bcb")
                nc.gpsimd.partition_broadcast(bc[:], rstd[:, islice].bitcast(F32))
                if o == 4:
                    dst = kn[Q]
                else:
                    dst = qnp.tile([D, 512], BF16, tag=f"qn{o}")
                    qn[(o, Q)] = dst
                nc.vector.tensor_tensor(
                    dst[:], qc_tiles[o][:], bc[:], mybir.AluOpType.mult
                )

            def phase_P(Q):
                qsl = slice(Q * 512, (Q + 1) * 512)
                qc_tiles = {}
                im_q = st_pool.tile([1, 4 * 512], F32, tag="imq")
                im_k = st_pool.tile([1, 512], F32, tag="imk")
                for pidx, pdef in enumerate(passes):
                    proj = [pp.tile([128, 512], F32, name=f"pp{i}") for i in range(3)]
                    for t in range(nT):
                        hst = wk_pool.tile([128, 512], BF16, tag="hst")
                        nc.sync.dma_start(
                            hst[:], hsT_d[t * 128:(t + 1) * 128, qsl]
                        )
                        for i, (kind, idx) in enumerate(pdef):
                            nc.tensor.matmul(
                                proj[i][:], w_slice(kind, idx, t), hst[:],
                                start=(t == 0), stop=(t == nT - 1),
                            )
                    for i, (kind, idx) in enumerate(pdef):
                        ps = proj[i]
                        if kind == "v":
                            vtsb = wk_pool.tile([128, 512], BF16, tag="vtsb")
                            nc.vector.tensor_copy(vtsb[:], ps[:])
                            for st in range(4):
                                vt_ps = pp.tile([128, 128], BF16, name="pm")
                                nc.tensor.transpose(
                                    vt_ps[:], vtsb[:, st * 128:(st + 1) * 128],
                                    idn_sb[:],
                                )
                                nc.vector.tensor_copy(vv[Q * 4 + st][:], vt_ps[:])
                        elif kind == "k":
                            rope_stats(Q, qsl, 4, ps, qc_tiles, im_k, slice(0, 512))
                        else:
                            o = idx
                            rope_stats(
                                Q, qsl, o, ps, qc_tiles, im_q,
                                slice(o * 512, (o + 1) * 512),
                            )
                    if pidx == 1:
                        rstd_q = st_pool.tile([1, 4 * 512], F32R, tag="rstdq")
                        nc.scalar.sqrt(rstd_q[:], im_q[:])
                        for o in range(4):
                            apply_norm(Q, o, rstd_q, slice(o * 512, (o + 1) * 512), qc_tiles)
                        rstd_k = st_pool.tile([1, 512], F32R, tag="rstdk")
                        nc.scalar.sqrt(rstd_k[:], im_k[:])
                        apply_norm(Q, 4, rstd_k, slice(0, 512), qc_tiles)

            def phase_A_h(Q, h):
                last_sj = Q * 4 + 3
                att = pp.tile([128, 512], F32, name="sat")
                ssum = pp.tile([1, 512], F32, name="ssm")
                for sj in range(last_sj + 1):
                    sc = pp.tile(
                        [128, 512], F32,
                        name=("sa0" if sj % 2 == 0 else "sa1"),
                    )
                    nc.tensor.matmul(
                        sc[:],
                        kn[sj // 4][:, (sj % 4) * 128:(sj % 4 + 1) * 128],
                        qn[(h, Q)][:],
                        start=True, stop=True,
                    )
                    t_sb = wk_pool.tile([128, 512], BF16, tag="tt")
                    nc.scalar.activation(
                        t_sb[:], sc[:], mybir.ActivationFunctionType.Tanh,
                        scale=scale_tanh,
                    )
                    if sj >= Q * 4:
                        nc.gpsimd.affine_select(
                            out=t_sb[:], in_=t_sb[:],
                            compare_op=mybir.AluOpType.is_ge,
                            fill=-1e9,
                            base=Q * 512 - sj * 128,
                            pattern=[[1, 512]],
                            channel_multiplier=-1,
                        )
                    e_sb = wk_pool.tile([128, 512], BF16, tag="ee")
                    nc.scalar.activation(
                        e_sb[:], t_sb[:], mybir.ActivationFunctionType.Exp,
                        scale=SOFT_CAP,
                    )
                    nc.tensor.matmul(
                        att[:], vv[sj][:], e_sb[:],
                        start=(sj == 0), stop=(sj == last_sj),
                    )
                    nc.tensor.matmul(
                        ssum[:], ones_col[:], e_sb[:],
                        start=(sj == 0), stop=(sj == last_sj),
                    )
                sf = st2_pool.tile([1, 512], F32, tag="sf")
                nc.vector.tensor_copy(sf[:], ssum[:])
                scr2 = st2_pool.tile([1, 512], F32, tag="scr")
                rf = st2_pool.tile([1, 512], F32, tag="rf")
                nc.vector.reciprocal_approx_accurate(rf[:], sf[:], scr2[:])
                bcr_sb = wk_pool.tile([128, 512], F32, tag="evac")
                nc.gpsimd.partition_broadcast(bcr_sb[:], rf[:])
                at_t = atp.tile([D, 512], BF16, tag=f"at{h}")
                at[(h, Q)] = at_t
                nc.vector.tensor_tensor(
                    at_t[:], att[:], bcr_sb[:], mybir.AluOpType.mult
                )

            def phase_O(Q):
                for st in range(4):
                    row0 = Q * 512 + st * 128
                    for hb in range(4):
                        po = pp.tile([128, 512], F32, name="sa1")
                        for h in range(HQ):
                            nc.tensor.matmul(
                                po[:], at[(h, Q)][:, st * 128:(st + 1) * 128],
                                wo_sb[:, h, hb * 512:(hb + 1) * 512],
                                start=(h == 0), stop=(h == HQ - 1),
                            )
                        ob = wk_pool.tile([128, 512], F32, tag="evac")
                        nc.vector.tensor_copy(ob[:], po[:])
                        nc.sync.dma_start(
                            out_d[row0:row0 + 128, hb * 512:(hb + 1) * 512], ob[:]
                        )

            for Q in range(nQ):
                phase_P(Q)
                if Q > 0:
                    phase_O(Q - 1)
                for h in range(HQ):
                    phase_A_h(Q, h)
            phase_O(nQ - 1)

    nc.compile()
    return nc


def _get_nc(S):
    if S not in _BUILD_CACHE:
        _BUILD_CACHE[S] = _build(S)
    return _BUILD_CACHE[S]


def _rope_tables(S):
    inv_freq = 1.0 / (ROPE_BASE ** (np.arange(0, D, 2, dtype=np.float64) / D))
    pos = np.arange(S, dtype=np.float64)
    freqs = np.outer(pos, inv_freq)                  # [S, D/2]
    emb = np.concatenate([freqs, freqs], axis=-1)    # [S, D]
    return (
        np.cos(emb).T.astype(np.float32).copy(),     # [D, S]
        np.sin(emb).T.astype(np.float32).copy(),
    )


def _rot_matrix():
    R = np.zeros((D, D), dtype=np.float32)
    half = D // 2
    for i in range(half):
        R[i, i + half] = -1.0
        R[i + half, i] = 1.0
    return R


def run_sharded(hidden_states, Wq, Wk, Wv, Wo, q_norm_w, k_norm_w, trace=False):
    hidden_states = np.asarray(hidden_states, dtype=np.float32)
    Wq = np.asarray(Wq, dtype=np.float32)
    Wk = np.asarray(Wk, dtype=np.float32)
    Wv = np.asarray(Wv, dtype=np.float32)
    Wo = np.asarray(Wo, dtype=np.float32)
    q_norm_w = np.asarray(q_norm_w, dtype=np.float32)
    k_norm_w = np.asarray(k_norm_w, dtype=np.float32)

    B, S, _ = hidden_states.shape
    nc = _get_nc(S)

    cosT, sinT = _rope_tables(S)
    cosq = np.ascontiguousarray(cosT * q_norm_w[:, None])
    cosk = np.ascontiguousarray(cosT * k_norm_w[:, None])
    R = _rot_matrix()
    rwq = np.ascontiguousarray(R.T * q_norm_w[:, None])  # lhsT for rot-matmul
    rwk = np.ascontiguousarray(R.T * k_norm_w[:, None])
    idn = np.eye(D, dtype=np.float32)

    bf16 = ml_dtypes.bfloat16
    hsT = [np.ascontiguousarray(hidden_states[b].T).astype(bf16) for b in range(B)]

    in_maps = []
    for b in range(B):
        for g in range(NUM_KV_GROUPS):
            c0 = g * (NUM_HEADS // NUM_KV_GROUPS) * D
            c1 = (g + 1) * (NUM_HEADS // NUM_KV_GROUPS) * D
            in_maps.append({
                "hsT": hsT[b],
                "wq": np.ascontiguousarray(Wq[:, c0:c1]).astype(bf16),
                "wk": np.ascontiguousarray(Wk[:, g * D:(g + 1) * D]).astype(bf16),
                "wv": np.ascontiguousarray(Wv[:, g * D:(g + 1) * D]).astype(bf16),
                "wo": np.ascontiguousarray(Wo[c0:c1, :]).astype(bf16),
                "cosq": cosq,
                "cosk": cosk,
                "sin": sinT,
                "rwq": rwq,
                "rwk": rwk,
                "idn": idn.astype(bf16),
                "onesc": np.ones((128, 1), dtype=bf16),
            })

    res = run_bass_kernel_spmd(
        nc, in_maps, core_ids=list(range(len(in_maps))), trace=trace
    )

    out = np.zeros((B, S, HID), dtype=np.float64)
    for b in range(B):
        for g in range(NUM_KV_GROUPS):
            out[b] += res.results[b * NUM_KV_GROUPS + g]["out"].astype(np.float64)
    return out.astype(np.float32), res


def kernel(hidden_states, Wq, Wk, Wv, Wo, q_norm_w, k_norm_w):
    out, _ = run_sharded(hidden_states, Wq, Wk, Wv, Wo, q_norm_w, k_norm_w)
    return out
